# revision 46
# baseline (speedup 1.0000x reference)
"""Trainium2 Bass kernel: dense transformer block (B=2, S=2048, D=1024, H=16, DFF=4096).

Strategy: sequence-parallel across 8 NeuronCores (2 batches x 4 cores). Each core
owns 4 query-chunks of 128 tokens, interleaved {j, 7-j, 8+j, 15-j} so causal
attention work is balanced; per-core causal depth is padded to fixed slot budgets
(16, 12, 8, 4) with host-supplied 0/1 masks making the padding exact. K^T and V
are shared within each batch group via two AllGathers, staged in DRAM in exactly
the SBUF layout attention consumes (contiguous per-kblock tiles, softmax-sum
"ones" column embedded in V). All matmul operands are bf16 (PSUM accumulation
stays f32); residual adds use f32 copies. Wo@Wp is fused on the host; 1/sqrt(64)
is folded into Wq.
"""
import numpy as np

B, S, D, H, W, DFF = 2, 2048, 1024, 16, 64, 4096
N_CORES = 8
TOK = 512            # tokens per core
NKB = 16             # key blocks (of 128 tokens) per batch
HALVES = ((0, 8), (8, 16))   # kblock halves for the attention pass

_CACHE = {}


def _chunk_rank_slot(c):
    """Global 128-token chunk c (0..15) -> (group-rank, slot). Rank j owns
    chunks {j, 7-j, 8+j, 15-j}, stored in slot order sorted by causal depth
    descending: slots = [15-j, 8+j, 7-j, j]."""
    if c < 4:
        return c, 3
    if c < 8:
        return 7 - c, 2
    if c < 12:
        return c - 8, 1
    return 15 - c, 0


def _rank_chunks(j):
    """Slot s -> global chunk for group-rank j."""
    return [15 - j, 8 + j, 7 - j, j]


def _width(t):
    """Prefix width of valid q columns for kblock t (slot budgets 16/12/8/4)."""
    return 512 - 128 * (t // 4)


def _build_module(single=False, reps=1, nocc=False, phase_stop=3,
                  pair_exp=True, abl_no_exp=False, abl_no_mask=False):
    import concourse.bacc as bacc
    import concourse.tile as tile
    from concourse import mybir

    F32 = mybir.dt.float32
    F32R = mybir.dt.float32r
    BF16 = mybir.dt.bfloat16
    AF = mybir.ActivationFunctionType
    Alu = mybir.AluOpType

    nc = bacc.Bacc("TRN2", target_bir_lowering=False, debug=False,
                   num_devices=1 if single else N_CORES)

    # ---- per-core inputs ----
    xT_d = nc.dram_tensor("xT", [128, 8, TOK], F32, kind="ExternalInput").ap()
    xTh_d = nc.dram_tensor("xTh", [128, 8, TOK], BF16, kind="ExternalInput").ap()
    mask_d = nc.dram_tensor("mask", [128, NKB, 128], BF16,
                            kind="ExternalInput").ap()
    # ---- shared inputs (same data on every core) ----
    wq_d = nc.dram_tensor("wq", [128, 8, D], BF16, kind="ExternalInput").ap()
    wk_d = nc.dram_tensor("wk", [128, 8, D], BF16, kind="ExternalInput").ap()
    wv_d = nc.dram_tensor("wv", [128, 8, D], BF16, kind="ExternalInput").ap()
    wop_d = nc.dram_tensor("wop", [8, 128, 8, 128], BF16, kind="ExternalInput").ap()
    w1_d = nc.dram_tensor("w1", [32, 128, 8, 128], BF16,
                          kind="ExternalInput").ap()
    w2_d = nc.dram_tensor("w2", [8, 128, 32, 128], BF16,
                          kind="ExternalInput").ap()
    bq_d = nc.dram_tensor("bq", [128, 8], F32, kind="ExternalInput").ap()
    bk_d = nc.dram_tensor("bk", [128, 8], F32, kind="ExternalInput").ap()
    bop_d = nc.dram_tensor("bop", [128, 8], F32, kind="ExternalInput").ap()
    b1_d = nc.dram_tensor("b1", [128, 32], F32, kind="ExternalInput").ap()
    b2_d = nc.dram_tensor("b2", [128, 8], F32, kind="ExternalInput").ap()
    sel_d = nc.dram_tensor("sel", [2, 128], F32R, kind="ExternalInput").ap()
    ones_d = nc.dram_tensor("ones", [128, 16], BF16, kind="ExternalInput").ap()

    out_d = nc.dram_tensor("outT", [D, TOK], F32, kind="ExternalOutput").ap()

    groups = [[0, 1, 2, 3], [4, 5, 6, 7]]

    with tile.TileContext(nc) as tc:
      for _rep in range(reps):
          with (
              tc.tile_pool(name="const", bufs=1) as constp,
              tc.tile_pool(name="mid", bufs=1) as midp,
              tc.tile_pool(name="osb", bufs=3) as osbp,
              tc.tile_pool(name="dram", bufs=1, space="DRAM") as dramp,
          ):
              # ---------- persistent loads ----------
              # xTh first: it gates the first K-proj matmul. xT (f32) is only
              # needed at the output-projection residual, much later.
              xTh = constp.tile([128, 8, TOK], BF16, name="xTh")
              for k in range(8):
                  nc.sync.dma_start(xTh[:, k, :], xTh_d[:, k, :])
              # xT (f32) is only consumed by the output-projection residual;
              # its load is emitted right before the attention phase so the
              # 2MB transfer doesn't delay the K/V bounce writes and gathers
              # queued behind it on the sync ring.
              xT = constp.tile([128, 8, TOK], F32, name="xT")
              mask_t = constp.tile([128, NKB, 128], BF16, name="mask_t")
              sel2_t = constp.tile([2, 128], F32R, name="sel2_t")
              nc.sync.dma_start(sel2_t[:], sel_d)
              ones_t = constp.tile([128, 16], BF16, name="ones_t")
              nc.sync.dma_start(ones_t[:], ones_d)
              bq_t = constp.tile([128, 8], F32, name="bq_t")
              nc.sync.dma_start(bq_t[:], bq_d)
              bk_t = constp.tile([128, 8], F32, name="bk_t")
              nc.sync.dma_start(bk_t[:], bk_d)
              bop_t = constp.tile([128, 8], F32, name="bop_t")
              nc.sync.dma_start(bop_t[:], bop_d)
              b1_t = constp.tile([128, 32], F32, name="b1_t")
              nc.sync.dma_start(b1_t[:], b1_d)
              b2_t = constp.tile([128, 8], F32, name="b2_t")
              nc.sync.dma_start(b2_t[:], b2_d)

              # cross-phase tiles
              qT = constp.tile([128, 8, TOK], BF16, name="qT")
              attnT = midp.tile([128, 8, TOK], BF16, name="attnT")
              hresT = midp.tile([128, 8, TOK], F32, name="hresT")
              hresTh = midp.tile([128, 8, TOK], BF16, name="hresTh")

              # DRAM bounce + gather buffers for the collectives, split in two
              # halves (A = slots {3,2} feeding attention half0, B = slots
              # {1,0} feeding half1) so half0 K/V loads can start while the
              # B gathers are still in flight. Layouts match the SBUF tiles
              # attention loads:
              #   K: [slotpos, feat128, dchunk8, key128]
              #   V: [slotpos, key128, dchunk8, 130]  (per dchunk:
              #      [V_h_even(64) | 1 | V_h_odd(64) | 1] for the softmax sum)
              kt_dramA = dramp.tile([2, 128, 8, 128], BF16, name="kt_dramA")
              kt_dramB = dramp.tile([2, 128, 8, 128], BF16, name="kt_dramB")
              ktg_dramA = dramp.tile([4, 2, 128, 8, 128], BF16,
                                     name="ktg_dramA")
              ktg_dramB = dramp.tile([4, 2, 128, 8, 128], BF16,
                                     name="ktg_dramB")
              v_dramA = dramp.tile([2, 128, 8, 130], BF16, name="v_dramA")
              v_dramB = dramp.tile([2, 128, 8, 130], BF16, name="v_dramB")
              vg_dramA = dramp.tile([4, 2, 128, 8, 130], BF16,
                                    name="vg_dramA")
              vg_dramB = dramp.tile([4, 2, 128, 8, 130], BF16,
                                    name="vg_dramB")

              def slot_buf(s):
                  """Slot s -> (bounceK, gatherK, bounceV, gatherV, pos)."""
                  if s >= 2:
                      return (kt_dramA, ktg_dramA, v_dramA, vg_dramA, 3 - s)
                  return (kt_dramB, ktg_dramB, v_dramB, vg_dramB, 1 - s)

              # ---------- QKV projections ----------
              with (
                  tc.tile_pool(name="wproj", bufs=2) as wprojp,
                  tc.tile_pool(name="kvout", bufs=1) as kvoutp,
                  tc.tile_pool(name="vsb", bufs=2) as vsbp,
                  tc.tile_pool(name="ps_a", bufs=6, space="PSUM") as psa,
              ):
                  # PE warm-up: dummy matmuls on whatever is in SBUF while the
                  # first input DMAs land. Keeps the HAM activity window busy
                  # so the real projections start at 2.4 GHz instead of 1.2,
                  # and fills the otherwise-idle load gap. Output is discarded.
                  warm = kvoutp.tile([128, 512], BF16, name="warm")
                  nc.vector.memset(warm[:], 0.0)
                  wpp = psa.tile([128, TOK], F32, name="wpp", tag="psa")
                  for i in range(10):
                      nc.tensor.matmul(wpp[:], warm[:, 0:128], warm[:],
                                       start=(i == 0), stop=(i == 9))
                  # ACT table pre-load: a dummy exp here pulls the
                  # exp_and_others table set in while ScalarE is idle, so the
                  # first real exp at attention start doesn't stall ~2.7us on
                  # the table DMA. Identity (phase 1 biases) is in every set,
                  # so no extra switch is introduced.
                  dummy = kvoutp.tile([128, 1], BF16, name="dummy")
                  nc.vector.memset(dummy[:], 0.0)
                  nc.scalar.activation(dummy[:], dummy[:], AF.Exp)

                  # K^T -> slot-major DRAM -> AllGather
                  wk_t = wprojp.tile([128, 8, D], BF16, name="wk_t",
                                     tag="wproj")
                  for k in range(8):
                      nc.scalar.dma_start(wk_t[:, k, :], wk_d[:, k, :])
                  kt_full = kvoutp.tile([128, 8, TOK], BF16, name="kt_full")
                  for m in range(8):
                      pp = psa.tile([128, TOK], F32, name="pp_k", tag="psa")
                      for k in range(8):
                          nc.tensor.matmul(
                              pp[:], wk_t[:, k, m * 128:(m + 1) * 128],
                              xTh[:, k, :], start=(k == 0), stop=(k == 7))
                      nc.scalar.activation(kt_full[:, m, :], pp[:],
                                           AF.Identity, bias=bk_t[:, m:m + 1])
                  for s in (3, 2, 1, 0):
                      kb, kg, _, _, pos = slot_buf(s)
                      if single or nocc:
                          for r in range(4):
                              nc.sync.dma_start(
                                  kg[r, pos],
                                  kt_full[:, :, s * 128:(s + 1) * 128])
                      else:
                          nc.sync.dma_start(
                              kb[pos], kt_full[:, :, s * 128:(s + 1) * 128])
                      if not single and not nocc and s in (2, 0):
                          nc.gpsimd.collective_compute(
                              "AllGather", Alu.bypass, replica_groups=groups,
                              ins=[(kt_dramA if s == 2 else kt_dramB).opt()],
                              outs=[(ktg_dramA if s == 2
                                     else ktg_dramB).opt()])

                  # V -> interleaved DRAM (with ones column) -> AllGather
                  wv_t = wprojp.tile([128, 8, D], BF16, name="wv_t",
                                     tag="wproj")
                  for k in range(8):
                      nc.gpsimd.dma_start(wv_t[:, k, :], wv_d[:, k, :])
                  for tc4 in (3, 2, 1, 0):
                      v_sb = vsbp.tile([128, 8, 130], BF16, name="v_sb",
                                       tag="vsb")
                      nc.vector.tensor_copy(
                          v_sb.rearrange("p m (hh z) -> p m hh z", hh=2)
                          [:, :, :, 64:65],
                          ones_t.rearrange("p (m hh) -> p m hh", m=8))
                      for half in range(2):
                          pp = psa.tile([128, 512], F32, name="pp_v",
                                        tag="psa")
                          for k in range(8):
                              nc.tensor.matmul(
                                  pp[:], xTh[:, k, tc4 * 128:(tc4 + 1) * 128],
                                  wv_t[:, k, half * 512:(half + 1) * 512],
                                  start=(k == 0), stop=(k == 7))
                          nc.vector.tensor_copy(
                              v_sb[:, half * 4:(half + 1) * 4, :]
                              .rearrange("p m (hh z) -> p m hh z", hh=2)
                              [:, :, :, 0:64],
                              pp.rearrange("p (m hh w) -> p m hh w",
                                           m=4, hh=2))
                      _, _, vb, vg, pos = slot_buf(tc4)
                      if single or nocc:
                          for r in range(4):
                              nc.sync.dma_start(vg[r, pos], v_sb[:])
                      else:
                          nc.sync.dma_start(vb[pos], v_sb[:])
                      if not single and not nocc and tc4 in (2, 0):
                          nc.gpsimd.collective_compute(
                              "AllGather", Alu.bypass, replica_groups=groups,
                              ins=[(v_dramA if tc4 == 2 else v_dramB).opt()],
                              outs=[(vg_dramA if tc4 == 2
                                     else vg_dramB).opt()])

                  # Q^T (host folded 1/8 into wq/bq)
                  wq_t = wprojp.tile([128, 8, D], BF16, name="wq_t",
                                     tag="wproj")
                  for k in range(8):
                      nc.gpsimd.dma_start(wq_t[:, k, :], wq_d[:, k, :])
                  for m in range(8):
                      pp = psa.tile([128, TOK], F32, name="pp_q", tag="psa")
                      for k in range(8):
                          nc.tensor.matmul(
                              pp[:], wq_t[:, k, m * 128:(m + 1) * 128],
                              xTh[:, k, :], start=(k == 0), stop=(k == 7))
                      nc.scalar.activation(qT[:, m, :], pp[:], AF.Identity,
                                           bias=bq_t[:, m:m + 1])

              if phase_stop == 1:
                  for m in range(8):
                      ob = osbp.tile([128, TOK], F32, name="ob1", tag="osb")
                      nc.vector.tensor_copy(ob[:], qT[:, m, :])
                      nc.sync.dma_start(out_d[m * 128:(m + 1) * 128, :], ob[:])
                  continue

              # deferred loads (see comment at the xT tile above): neither is
              # needed before attention, so keep the early sync ring clear
              # for the K/V bounce writes that gate the gathers.
              nc.sync.dma_start(mask_t[:], mask_d)
              nc.sync.dma_start(xT[:], xT_d)

              # ---------- attention ----------
              with (
                  tc.tile_pool(name="kv", bufs=2) as kvp,
                  tc.tile_pool(name="workB", bufs=4) as workB,
                  tc.tile_pool(name="attnu", bufs=1) as attnup,
                  tc.tile_pool(name="ps_st", bufs=2, space="PSUM") as psst,
                  tc.tile_pool(name="ps_pv", bufs=2, space="PSUM") as pspv,
                  tc.tile_pool(name="ps_bc", bufs=1, space="PSUM") as psbc,
              ):
                  attnU = attnup.tile([65, 16, TOK], F32, name="attnU")
                  for hi, (t0, t1) in enumerate(HALVES):
                      kth = {}
                      vth = {}
                      for t in range(t0, t1):
                          r, s = _chunk_rank_slot(t)
                          _, kg, _, vg, pos = slot_buf(s)
                          kt_t = kvp.tile([128, 8, 128], BF16,
                                          name=f"kt_{t}", tag=f"kth{t % 8}")
                          nc.sync.dma_start(kt_t[:], kg[r, pos])
                          kth[t] = kt_t
                          v_t = kvp.tile([128, 8, 130], BF16,
                                         name=f"v_{t}", tag=f"vth{t % 8}")
                          nc.sync.dma_start(v_t[:], vg[r, pos])
                          vth[t] = v_t
                      for h in range(16):
                          m2, h2 = h // 2, h % 2
                          pv = pspv.tile([65, TOK], F32, name="pv", tag="pv")

                          prs = {}

                          def do_pair(pi, m2=m2, h2=h2, kth=kth, prs=prs,
                                      t0=t0):
                              ta = t0 + 2 * pi
                              wt = _width(ta)
                              st2 = psst.tile([128, 2, 512], F32, name="st2",
                                              tag="st2")
                              for i in range(2):
                                  nc.tensor.matmul(
                                      st2[:, i, 0:wt],
                                      kth[ta + i][h2 * 64:(h2 + 1) * 64,
                                                  m2, :],
                                      qT[h2 * 64:(h2 + 1) * 64, m2, 0:wt],
                                      start=True, stop=True)
                              pr2 = workB.tile([128, 2, 512], BF16,
                                               name="pr2", tag="pr")
                              if abl_no_exp:
                                  nc.vector.tensor_copy(pr2[:, :, 0:wt],
                                                        st2[:, :, 0:wt])
                                  if not abl_no_mask:
                                      nc.vector.tensor_mul(
                                          pr2[:, :, wt - 128:wt],
                                          pr2[:, :, wt - 128:wt],
                                          mask_t[:, ta:ta + 2, :])
                              elif abl_no_mask:
                                  nc.scalar.activation(pr2[:, :, 0:wt],
                                                       st2[:, :, 0:wt],
                                                       AF.Exp)
                              elif pair_exp:
                                  nc.scalar.activation(pr2[:, :, 0:wt],
                                                       st2[:, :, 0:wt],
                                                       AF.Exp)
                                  nc.vector.tensor_mul(
                                      pr2[:, :, wt - 128:wt],
                                      pr2[:, :, wt - 128:wt],
                                      mask_t[:, ta:ta + 2, :])
                              else:
                                  for i in range(2):
                                      nc.scalar.activation(
                                          pr2[:, i, 0:wt], st2[:, i, 0:wt],
                                          AF.Exp)
                                      nc.vector.tensor_mul(
                                          pr2[:, i, wt - 128:wt],
                                          pr2[:, i, wt - 128:wt],
                                          mask_t[:, ta + i, :])
                              prs[pi] = pr2

                          # software pipeline: keep S^T/exp/mask 2 pairs ahead
                          do_pair(0)
                          do_pair(1)
                          for pi in range(4):
                              if pi + 2 < 4:
                                  do_pair(pi + 2)
                              ta = t0 + 2 * pi
                              wt = _width(ta)
                              pr2 = prs.pop(pi)
                              for i in range(2):
                                  nc.tensor.matmul(
                                      pv[:, 0:wt],
                                      vth[ta + i][:, m2,
                                                  h2 * 65:h2 * 65 + 65],
                                      pr2[:, i, 0:wt],
                                      start=(pi == 0 and i == 0),
                                      stop=(pi == 3 and i == 1))
                          # pv only has valid data in the widest prefix of
                          # this half; beyond that is unwritten PSUM
                          wmax = _width(t0)
                          if hi == 0:
                              nc.vector.tensor_copy(attnU[:, h, 0:wmax],
                                                    pv[:, 0:wmax])
                          else:
                              nc.vector.tensor_add(attnU[:, h, 0:wmax],
                                                   attnU[:, h, 0:wmax],
                                                   pv[:, 0:wmax])
                              if h % 2 == 1:
                                  # normalize finished head pair (h-1, h)
                                  m = h // 2
                                  sums2 = attnup.tile(
                                      [2, TOK], F32, name="sums2",
                                      tag="sums2", bufs=3)
                                  nc.gpsimd.dma_start(
                                      sums2[:],
                                      attnU[64:65, h - 1:h + 1, :])
                                  recip2 = workB.tile([2, TOK], F32R,
                                                      name="recip2",
                                                      tag="recip2")
                                  with nc.allow_low_precision(
                                          reason="f32r softmax recip"):
                                      nc.vector.reciprocal(recip2[:],
                                                           sums2[:])
                                  bc = psbc.tile([128, TOK], F32,
                                                 name="bc", tag="bc")
                                  nc.tensor.matmul(bc[:], sel2_t[:],
                                                   recip2[:], start=True,
                                                   stop=True)
                                  for hh in range(2):
                                      nc.vector.tensor_mul(
                                          attnT[hh * 64:(hh + 1) * 64,
                                                m, :],
                                          attnU[0:64, 2 * m + hh, :],
                                          bc[hh * 64:(hh + 1) * 64, :])

              if phase_stop == 2:
                  for m in range(8):
                      ob = osbp.tile([128, TOK], F32, name="ob2", tag="osb")
                      nc.vector.tensor_copy(ob[:], attnT[:, m, :])
                      nc.sync.dma_start(out_d[m * 128:(m + 1) * 128, :], ob[:])
                  continue

              # ---------- output projection (Wo@Wp fused) + residual ----------
              with (
                  tc.tile_pool(name="wop", bufs=4) as wopp,
                  tc.tile_pool(name="ps_b", bufs=6, space="PSUM") as psb,
              ):
                  for m in range(8):
                      wop_t = wopp.tile([128, 8, 128], BF16, name="wop_t",
                                        tag="wopc")
                      nc.gpsimd.dma_start(wop_t[:], wop_d[m])
                      pp = psb.tile([128, TOK], F32, name="pp_o", tag="psb")
                      for k in range(8):
                          nc.tensor.matmul(
                              pp[:], wop_t[:, k, :],
                              attnT[:, k, :], start=(k == 0), stop=(k == 7))
                      nc.vector.scalar_tensor_tensor(
                          hresT[:, m, :], pp[:], bop_t[:, m:m + 1], xT[:, m, :],
                          op0=Alu.add, op1=Alu.add)
                      nc.vector.tensor_copy(hresTh[:, m, :], hresT[:, m, :])

                  # ---------- FFN ----------
                  with (
                      tc.tile_pool(name="gelu", bufs=1) as gelup,
                      tc.tile_pool(name="wffn", bufs=6) as wffnp,
                  ):
                      geluT = gelup.tile([128, 32, TOK], BF16, name="geluT")
                      for nf in range(32):
                          w1_t = wffnp.tile([128, 8, 128], BF16, name="w1_t",
                                            tag="w1")
                          nc.gpsimd.dma_start(w1_t[:], w1_d[nf])
                          pp = psb.tile([128, TOK], F32, name="pp_f1",
                                        tag="psb")
                          for k in range(8):
                              nc.tensor.matmul(pp[:], w1_t[:, k, :],
                                               hresTh[:, k, :],
                                               start=(k == 0), stop=(k == 7))
                          nc.scalar.activation(geluT[:, nf, :], pp[:], AF.Gelu,
                                               bias=b1_t[:, nf:nf + 1])
                      for m in range(8):
                          w2_t = wffnp.tile([128, 32, 128], BF16, name="w2_t",
                                            tag="w2", bufs=2)
                          nc.gpsimd.dma_start(w2_t[:], w2_d[m])
                          pp = psb.tile([128, TOK], F32, name="pp_f2",
                                        tag="psb")
                          for kf in range(32):
                              nc.tensor.matmul(pp[:], w2_t[:, kf, :],
                                               geluT[:, kf, :],
                                               start=(kf == 0),
                                               stop=(kf == 31))
                          out_sb = osbp.tile([128, TOK], F32, name="out_sb",
                                             tag="osb")
                          nc.vector.scalar_tensor_tensor(
                              out_sb[:], pp[:], b2_t[:, m:m + 1],
                              hresT[:, m, :], op0=Alu.add, op1=Alu.add)
                          nc.sync.dma_start(out_d[m * 128:(m + 1) * 128, :],
                                            out_sb[:])

    nc.compile()
    return nc


def _get_module():
    if "nc" not in _CACHE:
        _CACHE["nc"] = _build_module()
    return _CACHE["nc"]


def _bf16(a):
    import ml_dtypes
    return np.ascontiguousarray(a.astype(ml_dtypes.bfloat16))


def _prep_shared(Wq, bq, Wk, bk, Wv, bv, Wo, bo, Wp, bp, W1, b1, W2, b2):
    """Host-side weight preprocessing (fp32 in, blocked bf16/f32 out)."""
    Wq_s = (Wq.astype(np.float64) * 0.125).astype(np.float32)
    bq_s = (bq.astype(np.float64) * 0.125).astype(np.float32)
    Wop = (Wo.astype(np.float64) @ Wp.astype(np.float64)).astype(np.float32)
    bop = (bv.astype(np.float64) @ Wo.astype(np.float64) @ Wp.astype(np.float64)
           + bo.astype(np.float64) @ Wp.astype(np.float64)
           + bp.astype(np.float64)).astype(np.float32)
    return {
        "wq": _bf16(Wq_s.reshape(8, 128, D).transpose(1, 0, 2)),
        "wk": _bf16(Wk.reshape(8, 128, D).transpose(1, 0, 2)),
        "wv": _bf16(Wv.reshape(8, 128, D).transpose(1, 0, 2)),
        "wop": _bf16(Wop.reshape(8, 128, 8, 128).transpose(2, 1, 0, 3)),
        "w1": _bf16(W1.reshape(8, 128, 32, 128).transpose(2, 1, 0, 3)),
        "w2": _bf16(W2.reshape(32, 128, 8, 128).transpose(2, 1, 0, 3)),
        "bq": np.ascontiguousarray(bq_s.reshape(8, 128).T),
        "bk": np.ascontiguousarray(bk.reshape(8, 128).T),
        "bop": np.ascontiguousarray(bop.reshape(8, 128).T),
        "b1": np.ascontiguousarray(b1.reshape(32, 128).T),
        "b2": np.ascontiguousarray(b2.reshape(8, 128).T),
        "sel": np.ascontiguousarray(
            (np.arange(128)[None, :] // 64 == np.arange(2)[:, None])
            .astype(np.float32)),
        "ones": _bf16(np.ones((128, 16), np.float32)),
    }


def _prep_core(x, core):
    """Per-core inputs: xT (feature-major, slot order) and causal mask."""
    b, j = core // 4, core % 4
    chunks = _rank_chunks(j)
    xc = np.concatenate(
        [x[b, c * 128:(c + 1) * 128, :] for c in chunks], axis=0)  # [512, D]
    xT = np.ascontiguousarray(
        xc.T.reshape(8, 128, TOK).transpose(1, 0, 2))  # [128, 8, TOK]
    mask = np.zeros((NKB, 128, 128), np.float32)
    ki = np.arange(128)[:, None]
    qi = np.arange(128)[None, :]
    for t in range(NKB):
        s = 3 - t // 4
        c = chunks[s]
        mask[t] = ((c * 128 + qi) >= (t * 128 + ki)).astype(np.float32)
    return {"xT": xT, "xTh": _bf16(xT),
            "mask": _bf16(mask.transpose(1, 0, 2))}


def kernel(x, Wq, bq, Wk, bk, Wv, bv, Wo, bo, Wp, bp, W1, b1, W2, b2):
    from concourse.bass_utils import run_bass_kernel_spmd

    x = np.asarray(x, np.float32)
    shared = _prep_shared(np.asarray(Wq), np.asarray(bq), np.asarray(Wk),
                          np.asarray(bk), np.asarray(Wv), np.asarray(bv),
                          np.asarray(Wo), np.asarray(bo), np.asarray(Wp),
                          np.asarray(bp), np.asarray(W1), np.asarray(b1),
                          np.asarray(W2), np.asarray(b2))
    in_maps = []
    for c in range(N_CORES):
        m = dict(shared)
        m.update(_prep_core(x, c))
        in_maps.append(m)

    nc = _get_module()
    res = run_bass_kernel_spmd(nc, in_maps, core_ids=list(range(N_CORES)))
    _CACHE["last_results"] = res

    out = np.empty((B, S, D), np.float32)
    for c in range(N_CORES):
        b, j = c // 4, c % 4
        chunks = _rank_chunks(j)
        outT = res.results[c]["outT"]  # [D, 512]
        for s, ch in enumerate(chunks):
            out[b, ch * 128:(ch + 1) * 128, :] = \
                outT[:, s * 128:(s + 1) * 128].T
    return out
